# revision 5
# baseline (speedup 1.0000x reference)
"""Epipolar attention kernel for Trainium2 (8 NeuronCores, batch-parallel).

Host does the O(B) 3x3 geometry (SVD etc.) in float32 numpy, mirroring the
reference op-for-op; the device does all O(N^2) / O(N^2*C) work:
  d5[i,j]   = |5*(A_j*x_i + B_j*y_i + C_j)|        (PE, exact triple-bf16 split)
  e[i,j]    = exp(d5 - rowmax(d5)), r_i = rowsum   (ACT, fused accum)
  E2[i,j]   = exp(-e/r)                            (ACT, per-partition scale)
  attnT     = E2^T / colsum(E2)                    (PE transpose + ACT scale)
  out[i,c]  = sum_j attnT[j,i] * fsrcT[j,c]        (PE, fp16)
The double softmax identity: softmax_i(1 - p) == softmax_i(-p) == E2/colsum.
"""

import numpy as np
import ml_dtypes

import concourse.bass as bass
import concourse.bacc as bacc
import concourse.tile as tile
from concourse import mybir
from concourse.bass_utils import run_bass_kernel_spmd
from concourse.masks import make_identity

B, C, H, W = 8, 1152, 32, 32
N = H * W           # 1024
P = 128
NT = N // P         # 8
F32 = mybir.dt.float32
F16 = mybir.dt.float16
BF16 = mybir.dt.bfloat16
BFNP = ml_dtypes.bfloat16

TRACE = False
LAST_RESULTS = None


# ----------------------------------------------------------------- device ---

def _build_nc():
    nc = bacc.Bacc()
    fsrcT = nc.dram_tensor("fsrcT", (N, C), F16, kind="ExternalInput")
    abc9 = nc.dram_tensor("abc9", (9, N), BF16, kind="ExternalInput")
    xy9 = nc.dram_tensor("xy9", (9, N), BF16, kind="ExternalInput")
    out = nc.dram_tensor("out", (N, C), F32, kind="ExternalOutput")

    AF = mybir.ActivationFunctionType
    AO = mybir.AluOpType

    I32 = mybir.dt.int32

    with tile.TileContext(nc) as tc:
        with (
            tc.tile_pool(name="consts", bufs=1) as consts,
            tc.tile_pool(name="persist", bufs=1) as persist,
            tc.tile_pool(name="work", bufs=2) as work,
            tc.tile_pool(name="stats", bufs=4) as stats,
            tc.tile_pool(name="psA", bufs=2, space="PSUM") as psA,
            tc.tile_pool(name="psO", bufs=1, space="PSUM") as psO,
        ):
            xy_sb = consts.tile([9, N], BF16, tag="xy")
            nc.sync.dma_start(out=xy_sb, in_=xy9[:, :])
            abc_sb = consts.tile([9, N], BF16, tag="abc")
            nc.sync.dma_start(out=abc_sb, in_=abc9[:, :])

            fs_sb = persist.tile([P, NT, C], F16, tag="fs")
            for j in range(NT):
                nc.sync.dma_start(out=fs_sb[:, j, :], in_=fsrcT[j * P:(j + 1) * P, :])
            e2_sb = persist.tile([P, NT, N], F16, tag="e2")
            e2t_sb = persist.tile([P, NT, N], F16, tag="e2t")
            at_sb = persist.tile([P, NT, N], F16, tag="at")

            # Phase A: rows i on partitions, j on free dim
            for it in range(NT):
                d_ps = psA.tile([P, N], F32)
                for h in range(2):
                    nc.tensor.matmul(
                        d_ps[:, h * 512:(h + 1) * 512],
                        lhsT=xy_sb[:, it * P:(it + 1) * P],
                        rhs=abc_sb[:, h * 512:(h + 1) * 512],
                        start=True, stop=True,
                    )
                dabs = work.tile([P, N], F32, tag="dabs")
                nc.vector.tensor_scalar(
                    out=dabs.bitcast(I32), in0=d_ps.bitcast(I32),
                    scalar1=0x7FFFFFFF, scalar2=None, op0=AO.bitwise_and,
                )
                mx = stats.tile([P, 1], F32, tag="mx")
                nc.vector.tensor_reduce(
                    out=mx, in_=d_ps, axis=mybir.AxisListType.X, op=AO.max,
                    apply_absolute_value=True,
                )
                nmx = stats.tile([P, 1], F32, tag="nmx")
                nc.vector.tensor_scalar_mul(nmx, mx, -1.0)
                e_t = work.tile([P, N], F32, tag="e")
                r = stats.tile([P, 1], F32, tag="r")
                nc.scalar.activation(
                    out=e_t, in_=dabs, func=AF.Exp, bias=nmx, scale=1.0, accum_out=r
                )
                negr = stats.tile([P, 1], F32, tag="negr")
                nc.vector.tensor_scalar_mul(negr, r, -1.0)
                ninvr = stats.tile([P, 1], F32, tag="ninvr")
                nc.vector.reciprocal(ninvr, negr)     # -1/r
                nc.scalar.activation(
                    out=e2_sb[:, it, :], in_=e_t, func=AF.Exp, bias=0.0, scale=ninvr
                )
                # transpose this i-tile's blocks into all j-stripes (DMA xbar)
                for u in range(NT):
                    nc.sync.dma_start_transpose(
                        e2t_sb[:, u, it * P:(it + 1) * P],
                        e2_sb[:, it, u * P:(u + 1) * P],
                    )

            # Phase B: per j-stripe column sums + scale
            for u in range(NT):
                S = stats.tile([P, 1], F32, tag="S")
                nc.vector.tensor_reduce(
                    out=S, in_=e2t_sb[:, u, :], axis=mybir.AxisListType.X, op=AO.add
                )
                invS = stats.tile([P, 1], F32, tag="invS")
                nc.vector.reciprocal(invS, S)
                nc.vector.tensor_scalar_mul(at_sb[:, u, :], e2t_sb[:, u, :], invS)

            # Phase C: out[i,c] = sum_j attnT[j,i] * fsrcT[j,c]
            for it in range(NT):
                ops = psO.tile([P, C], F32)
                for j in range(NT):
                    for c0, cw in ((0, 512), (512, 512), (1024, 128)):
                        nc.tensor.matmul(
                            ops[:, c0:c0 + cw],
                            lhsT=at_sb[:, j, it * P:(it + 1) * P],
                            rhs=fs_sb[:, j, c0:c0 + cw],
                            start=(j == 0), stop=(j == NT - 1),
                        )
                osb = work.tile([P, C], F32, tag="osb")
                nc.scalar.copy(osb, ops)
                nc.sync.dma_start(out=out[it * P:(it + 1) * P, :], in_=osb)
    nc.compile()
    return nc


_NC = None


def _get_nc():
    global _NC
    if _NC is None:
        _NC = _build_nc()
    return _NC


# ------------------------------------------------------------------- host ---

def _skew(t):
    z = np.zeros_like(t[:, 0])
    return np.stack([
        np.stack([z, -t[:, 2], t[:, 1]], -1),
        np.stack([t[:, 2], z, -t[:, 0]], -1),
        np.stack([-t[:, 1], t[:, 0], z], -1),
    ], 1)


def _fundamental(K1, K2, R, t):
    E = _skew(t) @ R
    U, S, Vt = np.linalg.svd(E)
    S = S.copy()
    S[:, 2] = 0.0
    E = U @ (S[:, :, None] * Vt)
    return np.linalg.inv(np.swapaxes(K2, 1, 2)) @ E @ np.linalg.inv(K1)


def _split3(v):
    """Exact-ish triple bf16 split: v ~= hi + mid + lo (24 mantissa bits)."""
    v = v.astype(np.float32)
    hi = v.astype(BFNP)
    r1 = v - hi.astype(np.float32)
    mid = r1.astype(BFNP)
    r2 = r1 - mid.astype(np.float32)
    lo = r2.astype(BFNP)
    return hi, mid, lo


def _host_prep(f_src, K1, K2, R, t):
    ix, iy = np.meshgrid(np.arange(H, dtype=np.float32),
                         np.arange(W, dtype=np.float32), indexing="ij")
    comb = np.stack([ix.ravel(), iy.ravel(), np.ones(N, np.float32)], 0)  # (3,N)

    F = _fundamental(K1, K2, R, t)                    # (B,3,3)
    lines = (F @ comb).astype(np.float32)             # (B,3,N)
    lines = lines / lines[:, 2:3, :]
    y0 = -lines[:, 2, :] / lines[:, 1, :]
    y1 = -(lines[:, 2, :] + lines[:, 0, :] * np.float32(W)) / lines[:, 1, :]
    dy = y0 - y1
    L = np.sqrt(np.float32(W * W) + dy * dy)
    A5 = np.float32(5.0) * (dy / L)
    B5 = np.float32(5.0) * (np.float32(W) / L)
    C5 = np.float32(-5.0) * (np.float32(W) * y0 / L)

    Ah, Am, Al = _split3(A5)
    Bh, Bm, Bl = _split3(B5)
    Ch, Cm, Cl = _split3(C5)
    abc9 = np.stack([Ah, Bh, Ch, Am, Bm, Cm, Al, Bl, Cl], axis=1)  # (B,9,N) bf16
    xy9 = np.tile(comb, (3, 1)).astype(BFNP)                        # (9,N) exact

    fsT = np.ascontiguousarray(
        f_src.reshape(B, C, N).transpose(0, 2, 1)).astype(np.float16)  # (B,N,C)
    return abc9, xy9, fsT


def kernel(f_tar=None, f_src=None, K1=None, K2=None, R=None, t=None):
    global LAST_RESULTS
    f_src = np.asarray(f_src, np.float32)
    K1 = np.asarray(K1, np.float32)
    K2 = np.asarray(K2, np.float32)
    R = np.asarray(R, np.float32)
    t = np.asarray(t, np.float32)

    abc9, xy9, fsT = _host_prep(f_src, K1, K2, R, t)
    in_maps = [
        {"fsrcT": fsT[b], "abc9": np.ascontiguousarray(abc9[b]), "xy9": xy9}
        for b in range(B)
    ]
    res = run_bass_kernel_spmd(_get_nc(), in_maps, list(range(B)), trace=TRACE)
    LAST_RESULTS = res
    outs = np.stack([res.results[b]["out"] for b in range(B)], 0)  # (B,N,C)
    return outs.reshape(B, C, H, W)


# revision 8
# speedup vs baseline: 1.3871x; 1.3871x over previous
"""Epipolar attention kernel for Trainium2 (8 NeuronCores, batch-parallel).

Host does the O(B) 3x3 geometry (SVD etc.) in float32 numpy, mirroring the
reference op-for-op; the device does all O(N^2) / O(N^2*C) work:
  d5[i,j]   = |5*(A_j*x_i + B_j*y_i + C_j)|        (PE, exact triple-bf16 split)
  e[i,j]    = exp(d5 - rowmax(d5)), r_i = rowsum   (ACT, fused accum)
  E2[i,j]   = exp(-e/r)                            (ACT, per-partition scale)
  attnT     = E2^T / colsum(E2)                    (PE transpose + ACT scale)
  out[i,c]  = sum_j attnT[j,i] * fsrcT[j,c]        (PE, fp16)
The double softmax identity: softmax_i(1 - p) == softmax_i(-p) == E2/colsum.
"""

import numpy as np
import ml_dtypes

import concourse.bass as bass
import concourse.bacc as bacc
import concourse.tile as tile
from concourse import mybir
from concourse.bass_utils import run_bass_kernel_spmd
from concourse.masks import make_identity

B, C, H, W = 8, 1152, 32, 32
N = H * W           # 1024
P = 128
NT = N // P         # 8
F32 = mybir.dt.float32
F16 = mybir.dt.float16
BF16 = mybir.dt.bfloat16
BFNP = ml_dtypes.bfloat16

TRACE = False
LAST_RESULTS = None


# ----------------------------------------------------------------- device ---

def _build_nc():
    nc = bacc.Bacc()
    fsrcT = nc.dram_tensor("fsrcT", (N, C), F16, kind="ExternalInput")
    abc9 = nc.dram_tensor("abc9", (9, N), BF16, kind="ExternalInput")
    xy9 = nc.dram_tensor("xy9", (9, N), BF16, kind="ExternalInput")
    out = nc.dram_tensor("out", (N, C), F32, kind="ExternalOutput")

    AF = mybir.ActivationFunctionType
    AO = mybir.AluOpType

    I32 = mybir.dt.int32

    with tile.TileContext(nc) as tc:
        with (
            tc.tile_pool(name="consts", bufs=1) as consts,
            tc.tile_pool(name="persist", bufs=1) as persist,
            tc.tile_pool(name="work", bufs=2) as work,
            tc.tile_pool(name="stats", bufs=4) as stats,
            tc.tile_pool(name="psA", bufs=1, space="PSUM") as psA,
            tc.tile_pool(name="psT", bufs=2, space="PSUM") as psT,
            tc.tile_pool(name="psO", bufs=1, space="PSUM") as psO,
        ):
            ident = consts.tile([P, P], F16, tag="ident")
            make_identity(nc, ident)
            xy_sb = consts.tile([9, N], BF16, tag="xy")
            nc.sync.dma_start(out=xy_sb, in_=xy9[:, :])
            abc_sb = consts.tile([9, N], BF16, tag="abc")
            nc.sync.dma_start(out=abc_sb, in_=abc9[:, :])

            fs_sb = persist.tile([P, NT, C], F16, tag="fs")
            for j in range(NT):
                nc.sync.dma_start(out=fs_sb[:, j, :], in_=fsrcT[j * P:(j + 1) * P, :])
            e2_sb = persist.tile([P, NT, N], F16, tag="e2")
            at_sb = persist.tile([P, NT, N], F16, tag="at")

            # Phase A: rows i on partitions, j on free dim
            for it in range(NT):
                d_ps = psA.tile([P, N], F32)
                for h in range(2):
                    nc.tensor.matmul(
                        d_ps[:, h * 512:(h + 1) * 512],
                        lhsT=xy_sb[:, it * P:(it + 1) * P],
                        rhs=abc_sb[:, h * 512:(h + 1) * 512],
                        start=True, stop=True,
                    )
                dabs = work.tile([P, N], F32, tag="dabs")
                nc.vector.tensor_scalar(
                    out=dabs.bitcast(I32), in0=d_ps.bitcast(I32),
                    scalar1=0x7FFFFFFF, scalar2=None, op0=AO.bitwise_and,
                )
                mx = stats.tile([P, 1], F32, tag="mx")
                nc.vector.tensor_reduce(
                    out=mx, in_=d_ps, axis=mybir.AxisListType.X, op=AO.max,
                    apply_absolute_value=True,
                )
                nmx = stats.tile([P, 1], F32, tag="nmx")
                nc.vector.tensor_scalar_mul(nmx, mx, -1.0)
                e_t = work.tile([P, N], F32, tag="e")
                r = stats.tile([P, 1], F32, tag="r")
                nc.scalar.activation(
                    out=e_t, in_=dabs, func=AF.Exp, bias=nmx, scale=1.0, accum_out=r
                )
                negr = stats.tile([P, 1], F32, tag="negr")
                nc.vector.tensor_scalar_mul(negr, r, -1.0)
                ninvr = stats.tile([P, 1], F32, tag="ninvr")
                nc.vector.reciprocal(ninvr, negr)     # -1/r
                nc.scalar.activation(
                    out=e2_sb[:, it, :], in_=e_t, func=AF.Exp, bias=0.0, scale=ninvr
                )

            # Phase B: PE-transpose E2 stripes (fp16 PSUM, 1 bank each), then
            # column sums + scale into attnT
            for u in range(NT):
                tp = psT.tile([P, N], F16)
                for it in range(NT):
                    nc.tensor.transpose(
                        tp[:, it * P:(it + 1) * P],
                        e2_sb[:, it, u * P:(u + 1) * P],
                        ident,
                    )
                S = stats.tile([P, 1], F32, tag="S")
                nc.vector.tensor_reduce(
                    out=S, in_=tp, axis=mybir.AxisListType.X, op=AO.add
                )
                invS = stats.tile([P, 1], F32, tag="invS")
                nc.vector.reciprocal(invS, S)
                nc.scalar.mul(at_sb[:, u, :], tp, invS)

            # Phase C: out[i,c] = sum_j attnT[j,i] * fsrcT[j,c]
            for it in range(NT):
                ops = psO.tile([P, C], F32)
                for j in range(NT):
                    for c0, cw in ((0, 512), (512, 512), (1024, 128)):
                        nc.tensor.matmul(
                            ops[:, c0:c0 + cw],
                            lhsT=at_sb[:, j, it * P:(it + 1) * P],
                            rhs=fs_sb[:, j, c0:c0 + cw],
                            start=(j == 0), stop=(j == NT - 1),
                        )
                osb = work.tile([P, C], F32, tag="osb")
                nc.scalar.copy(osb, ops)
                nc.sync.dma_start(out=out[it * P:(it + 1) * P, :], in_=osb)
    nc.compile()
    return nc


_NC = None


def _get_nc():
    global _NC
    if _NC is None:
        _NC = _build_nc()
    return _NC


# ------------------------------------------------------------------- host ---

def _skew(t):
    z = np.zeros_like(t[:, 0])
    return np.stack([
        np.stack([z, -t[:, 2], t[:, 1]], -1),
        np.stack([t[:, 2], z, -t[:, 0]], -1),
        np.stack([-t[:, 1], t[:, 0], z], -1),
    ], 1)


def _fundamental(K1, K2, R, t):
    E = _skew(t) @ R
    U, S, Vt = np.linalg.svd(E)
    S = S.copy()
    S[:, 2] = 0.0
    E = U @ (S[:, :, None] * Vt)
    return np.linalg.inv(np.swapaxes(K2, 1, 2)) @ E @ np.linalg.inv(K1)


def _split3(v):
    """Exact-ish triple bf16 split: v ~= hi + mid + lo (24 mantissa bits)."""
    v = v.astype(np.float32)
    hi = v.astype(BFNP)
    r1 = v - hi.astype(np.float32)
    mid = r1.astype(BFNP)
    r2 = r1 - mid.astype(np.float32)
    lo = r2.astype(BFNP)
    return hi, mid, lo


def _host_prep(f_src, K1, K2, R, t):
    ix, iy = np.meshgrid(np.arange(H, dtype=np.float32),
                         np.arange(W, dtype=np.float32), indexing="ij")
    comb = np.stack([ix.ravel(), iy.ravel(), np.ones(N, np.float32)], 0)  # (3,N)

    F = _fundamental(K1, K2, R, t)                    # (B,3,3)
    lines = (F @ comb).astype(np.float32)             # (B,3,N)
    lines = lines / lines[:, 2:3, :]
    y0 = -lines[:, 2, :] / lines[:, 1, :]
    y1 = -(lines[:, 2, :] + lines[:, 0, :] * np.float32(W)) / lines[:, 1, :]
    dy = y0 - y1
    L = np.sqrt(np.float32(W * W) + dy * dy)
    A5 = np.float32(5.0) * (dy / L)
    B5 = np.float32(5.0) * (np.float32(W) / L)
    C5 = np.float32(-5.0) * (np.float32(W) * y0 / L)

    Ah, Am, Al = _split3(A5)
    Bh, Bm, Bl = _split3(B5)
    Ch, Cm, Cl = _split3(C5)
    abc9 = np.stack([Ah, Bh, Ch, Am, Bm, Cm, Al, Bl, Cl], axis=1)  # (B,9,N) bf16
    xy9 = np.tile(comb, (3, 1)).astype(BFNP)                        # (9,N) exact

    fsT = np.ascontiguousarray(
        f_src.reshape(B, C, N).transpose(0, 2, 1)).astype(np.float16)  # (B,N,C)
    return abc9, xy9, fsT


def kernel(f_tar=None, f_src=None, K1=None, K2=None, R=None, t=None):
    global LAST_RESULTS
    f_src = np.asarray(f_src, np.float32)
    K1 = np.asarray(K1, np.float32)
    K2 = np.asarray(K2, np.float32)
    R = np.asarray(R, np.float32)
    t = np.asarray(t, np.float32)

    abc9, xy9, fsT = _host_prep(f_src, K1, K2, R, t)
    in_maps = [
        {"fsrcT": fsT[b], "abc9": np.ascontiguousarray(abc9[b]), "xy9": xy9}
        for b in range(B)
    ]
    res = run_bass_kernel_spmd(_get_nc(), in_maps, list(range(B)), trace=TRACE)
    LAST_RESULTS = res
    outs = np.stack([res.results[b]["out"] for b in range(B)], 0)  # (B,N,C)
    return outs.reshape(B, C, H, W)


# revision 10
# speedup vs baseline: 1.4333x; 1.0333x over previous
"""Epipolar attention kernel for Trainium2 (8 NeuronCores, batch-parallel).

Host does the O(B) 3x3 geometry (SVD etc.) in float32 numpy, mirroring the
reference op-for-op; the device does all O(N^2) / O(N^2*C) work:
  d5[i,j]   = |5*(A_j*x_i + B_j*y_i + C_j)|        (PE, exact triple-bf16 split)
  e[i,j]    = exp(d5 - rowmax(d5)), r_i = rowsum   (ACT, fused accum)
  E2[i,j]   = exp(-e/r)                            (ACT, per-partition scale)
  attnT     = E2^T / colsum(E2)                    (PE transpose + ACT scale)
  out[i,c]  = sum_j attnT[j,i] * fsrcT[j,c]        (PE, fp16)
The double softmax identity: softmax_i(1 - p) == softmax_i(-p) == E2/colsum.
"""

import numpy as np
import ml_dtypes

import concourse.bass as bass
import concourse.bacc as bacc
import concourse.tile as tile
from concourse import mybir
from concourse.bass_utils import run_bass_kernel_spmd
from concourse.masks import make_identity

B, C, H, W = 8, 1152, 32, 32
N = H * W           # 1024
P = 128
NT = N // P         # 8
F32 = mybir.dt.float32
F16 = mybir.dt.float16
BF16 = mybir.dt.bfloat16
BFNP = ml_dtypes.bfloat16

TRACE = False
LAST_RESULTS = None


# ----------------------------------------------------------------- device ---

def _build_nc():
    nc = bacc.Bacc()
    fsrcT = nc.dram_tensor("fsrcT", (N, C), F16, kind="ExternalInput")
    abc9 = nc.dram_tensor("abc9", (9, N), BF16, kind="ExternalInput")
    xy9 = nc.dram_tensor("xy9", (9, N), BF16, kind="ExternalInput")
    identD = nc.dram_tensor("ident", (P, P), F16, kind="ExternalInput")
    out = nc.dram_tensor("out", (N, C), F32, kind="ExternalOutput")

    AF = mybir.ActivationFunctionType
    AO = mybir.AluOpType

    I32 = mybir.dt.int32

    with tile.TileContext(nc) as tc:
        with (
            tc.tile_pool(name="consts", bufs=1) as consts,
            tc.tile_pool(name="persist", bufs=1) as persist,
            tc.tile_pool(name="work", bufs=2) as work,
            tc.tile_pool(name="stats", bufs=4) as stats,
            tc.tile_pool(name="psA", bufs=1, space="PSUM") as psA,
            tc.tile_pool(name="psT", bufs=2, space="PSUM") as psT,
            tc.tile_pool(name="psC", bufs=4, space="PSUM") as psC,
        ):
            ident = consts.tile([P, P], F16, tag="ident")
            nc.sync.dma_start(out=ident, in_=identD[:, :])
            xy_sb = consts.tile([9, N], BF16, tag="xy")
            nc.sync.dma_start(out=xy_sb, in_=xy9[:, :])
            abc_sb = consts.tile([9, N], BF16, tag="abc")
            nc.sync.dma_start(out=abc_sb, in_=abc9[:, :])

            fs_sb = persist.tile([P, NT, C], F16, tag="fs")
            for j in range(NT):
                nc.sync.dma_start(out=fs_sb[:, j, :], in_=fsrcT[j * P:(j + 1) * P, :])
            e2_sb = persist.tile([P, NT, N], F16, tag="e2")
            at_sb = persist.tile([P, NT, N], F16, tag="at")

            # Phase A: rows i on partitions, j on free dim
            for it in range(NT):
                d_ps = psA.tile([P, N], F32)
                for h in range(2):
                    nc.tensor.matmul(
                        d_ps[:, h * 512:(h + 1) * 512],
                        lhsT=xy_sb[:, it * P:(it + 1) * P],
                        rhs=abc_sb[:, h * 512:(h + 1) * 512],
                        start=True, stop=True,
                    )
                dabs = work.tile([P, N], F32, tag="dabs")
                nc.vector.tensor_scalar(
                    out=dabs.bitcast(I32), in0=d_ps.bitcast(I32),
                    scalar1=0x7FFFFFFF, scalar2=None, op0=AO.bitwise_and,
                )
                nmx = stats.tile([P, 1], F32, tag="nmx")
                nc.vector.tensor_reduce(
                    out=nmx, in_=d_ps, axis=mybir.AxisListType.X, op=AO.max,
                    apply_absolute_value=True, negate=True,
                )
                e_t = work.tile([P, N], F32, tag="e")
                r = stats.tile([P, 1], F32, tag="r")
                nc.scalar.activation(
                    out=e_t, in_=dabs, func=AF.Exp, bias=nmx, scale=1.0, accum_out=r
                )
                negr = stats.tile([P, 1], F32, tag="negr")
                nc.vector.tensor_scalar_mul(negr, r, -1.0)
                ninvr = stats.tile([P, 1], F32, tag="ninvr")
                nc.vector.reciprocal(ninvr, negr)     # -1/r
                nc.scalar.activation(
                    out=e2_sb[:, it, :], in_=e_t, func=AF.Exp, bias=0.0, scale=ninvr
                )

            # Phase B: PE-transpose E2 stripes (fp16 PSUM, 1 bank each), then
            # column sums + scale into attnT
            for u in range(NT):
                tp = psT.tile([P, N], F16)
                for it in range(NT):
                    nc.tensor.transpose(
                        tp[:, it * P:(it + 1) * P],
                        e2_sb[:, it, u * P:(u + 1) * P],
                        ident,
                    )
                S = stats.tile([P, 1], F32, tag="S")
                nc.vector.tensor_reduce(
                    out=S, in_=tp, axis=mybir.AxisListType.X, op=AO.add
                )
                invS = stats.tile([P, 1], F32, tag="invS")
                nc.vector.reciprocal(invS, S)
                nc.scalar.mul(at_sb[:, u, :], tp, invS)

            # Phase C: out[i,c] = sum_j attnT[j,i] * fsrcT[j,c]
            CCH = ((0, 512), (512, 512), (1024, 128))
            for it in range(NT):
                ocs = [psC.tile([P, cw], F32, tag="oc", name=f"oc_{it}_{ci}")
                       for ci, (c0, cw) in enumerate(CCH)]
                for j in range(NT):
                    for ck, (c0, cw) in enumerate(CCH):
                        nc.tensor.matmul(
                            ocs[ck][:, :],
                            lhsT=at_sb[:, j, it * P:(it + 1) * P],
                            rhs=fs_sb[:, j, c0:c0 + cw],
                            start=(j == 0), stop=(j == NT - 1),
                        )
                for ck, (c0, cw) in enumerate(CCH):
                    osb = work.tile([P, 512], F32, tag="osb")
                    nc.scalar.copy(osb[:, :cw], ocs[ck])
                    nc.sync.dma_start(
                        out=out[it * P:(it + 1) * P, c0:c0 + cw], in_=osb[:, :cw]
                    )
    nc.compile()
    return nc


_NC = None


def _get_nc():
    global _NC
    if _NC is None:
        _NC = _build_nc()
    return _NC


# ------------------------------------------------------------------- host ---

def _skew(t):
    z = np.zeros_like(t[:, 0])
    return np.stack([
        np.stack([z, -t[:, 2], t[:, 1]], -1),
        np.stack([t[:, 2], z, -t[:, 0]], -1),
        np.stack([-t[:, 1], t[:, 0], z], -1),
    ], 1)


def _fundamental(K1, K2, R, t):
    E = _skew(t) @ R
    U, S, Vt = np.linalg.svd(E)
    S = S.copy()
    S[:, 2] = 0.0
    E = U @ (S[:, :, None] * Vt)
    return np.linalg.inv(np.swapaxes(K2, 1, 2)) @ E @ np.linalg.inv(K1)


def _split3(v):
    """Exact-ish triple bf16 split: v ~= hi + mid + lo (24 mantissa bits)."""
    v = v.astype(np.float32)
    hi = v.astype(BFNP)
    r1 = v - hi.astype(np.float32)
    mid = r1.astype(BFNP)
    r2 = r1 - mid.astype(np.float32)
    lo = r2.astype(BFNP)
    return hi, mid, lo


def _host_prep(f_src, K1, K2, R, t):
    ix, iy = np.meshgrid(np.arange(H, dtype=np.float32),
                         np.arange(W, dtype=np.float32), indexing="ij")
    comb = np.stack([ix.ravel(), iy.ravel(), np.ones(N, np.float32)], 0)  # (3,N)

    F = _fundamental(K1, K2, R, t)                    # (B,3,3)
    lines = (F @ comb).astype(np.float32)             # (B,3,N)
    lines = lines / lines[:, 2:3, :]
    y0 = -lines[:, 2, :] / lines[:, 1, :]
    y1 = -(lines[:, 2, :] + lines[:, 0, :] * np.float32(W)) / lines[:, 1, :]
    dy = y0 - y1
    L = np.sqrt(np.float32(W * W) + dy * dy)
    A5 = np.float32(5.0) * (dy / L)
    B5 = np.float32(5.0) * (np.float32(W) / L)
    C5 = np.float32(-5.0) * (np.float32(W) * y0 / L)

    Ah, Am, Al = _split3(A5)
    Bh, Bm, Bl = _split3(B5)
    Ch, Cm, Cl = _split3(C5)
    abc9 = np.stack([Ah, Bh, Ch, Am, Bm, Cm, Al, Bl, Cl], axis=1)  # (B,9,N) bf16
    xy9 = np.tile(comb, (3, 1)).astype(BFNP)                        # (9,N) exact

    fsT = np.ascontiguousarray(
        f_src.reshape(B, C, N).transpose(0, 2, 1)).astype(np.float16)  # (B,N,C)
    return abc9, xy9, fsT


def kernel(f_tar=None, f_src=None, K1=None, K2=None, R=None, t=None):
    global LAST_RESULTS
    f_src = np.asarray(f_src, np.float32)
    K1 = np.asarray(K1, np.float32)
    K2 = np.asarray(K2, np.float32)
    R = np.asarray(R, np.float32)
    t = np.asarray(t, np.float32)

    abc9, xy9, fsT = _host_prep(f_src, K1, K2, R, t)
    ident = np.eye(P, dtype=np.float16)
    in_maps = [
        {"fsrcT": fsT[b], "abc9": np.ascontiguousarray(abc9[b]), "xy9": xy9,
         "ident": ident}
        for b in range(B)
    ]
    res = run_bass_kernel_spmd(_get_nc(), in_maps, list(range(B)), trace=TRACE)
    LAST_RESULTS = res
    outs = np.stack([res.results[b]["out"] for b in range(B)], 0)  # (B,N,C)
    return outs.reshape(B, C, H, W)
